# revision 5
# baseline (speedup 1.0000x reference)
"""JointCCSA loss kernel for 8 Trainium2 NeuronCores.

reference:
    dists = cdist(X, X)                                  (bs, bs)
    sa_loss = 0.5 * sum[ same_y & ds_lt ] dists / n_sa
    s_loss  = 0.5 * sum[ y_lt  & ds_lt ] relu(1 - dists) / n_s

Strategy (data-parallel over rows of X, 8 cores, 512 rows each):
  * d2(i,j) = ||xb_i - xb_j||^2 via ONE augmented bf16 matmul into PSUM:
      lhsT = [-2*Xb_loc ; 1 ; 1]  (K=514),  rhs = [Xb^T ; sq_hi ; sq_lo]
    with sq = sum(bf16(X)^2) (so the Gram matrix is the exact-squared-dist
    of the rounded points -> d2 >= -eps, no NaN from sqrt).
  * dist = Sqrt(d2 + (sq_i + c0)) on ScalarE straight from PSUM (bias is
    per-partition), c0 = 0.0625 guards fp32-accumulation noise on the diag.
  * The pair masks are rank-12: mask(i,j) = e_i^T M e_j with e = onehot of
    (y, ds) combo (4*3=12).  So the masked reductions become tiny matmuls:
      T_sa(r,j) = sum_i U_sa(i,r) * dist(i,j)      U_sa(i,(c,a)) = [y_i==c][ds_i<a]
      T_s (r,j) = sum_i U_s (i,r) * min(dist,1)    U_s (i,(c,a)) = [y_i<c][ds_i<a]
    (min(d,1) = 1 - relu(1-d), so  sum A_s*relu(1-d) = N_pairs - sum A_s*min(d,1))
  * Host gathers T[combo(j), j] (one-hot contraction -> exact diag exclusion)
    and sums across cores.  Output: np.array([sa_loss, s_loss], float32).
"""

import numpy as np
import ml_dtypes
from contextlib import ExitStack

import concourse.bass as bass
import concourse.tile as tile
from concourse import mybir
from concourse.vector_clock import ScopedClock
from concourse.bass_utils import run_bass_kernel_spmd

BS = 4096
D = 512
NCORES = 8
MLOC = BS // NCORES          # 512 rows per core
MCH = MLOC // 128            # 4 partition chunks of local rows
KCH = D // 128               # 4 contraction chunks of X dims
JC = 4                       # j-chunks of width 1024
JW = 1024
C0 = 0.0625                  # sqrt-safety bias added into sq_i
BF16 = ml_dtypes.bfloat16


# ---------------------------------------------------------------------------
# Patch: this walrus build allows only ONE sync-wait on a CTRL-type (Drain)
# instruction; Tile's final drain aggregates many.  Spread them over
# single-wait SP nops.
def _patched_drain_and_barrier(self, tick_clock, wait_clock):
    nc = self.nc
    coll = nc.sync.nop(nofuse=True, hint="drain_wait_collector")
    wait_clock.add_sem_waits(coll.ins, ScopedClock({None: tick_clock.global_clock}))
    si = coll.ins.sync_info
    waits = list(si.on_wait) if si is not None else []
    if len(waits) > 1:
        si.on_wait = [waits[0]]
        for w in waits[1:]:
            n = nc.sync.nop(nofuse=True, hint="drain_wait_extra")
            n.ins.sync_info = mybir.SyncInfo(on_wait=[w], on_update=[])
    nc.sync.drain()
    nc.all_engine_barrier()
    assert self.sems is not None
    popped = nc._tile_sem_poison_stack.pop()
    assert popped is self._sem_poison
    nc.clear_and_free_semaphores(list(self.sems.allocated().values()))
    nc.all_engine_barrier()


tile.TileContext._drain_and_barrier = _patched_drain_and_barrier


def _split_waits(nc, maxw=1):
    """Hoist extra sync-waits from every instruction onto same-engine NoOps
    (this walrus build rejects instructions with more than ~1 wait)."""
    for fn in nc.m.functions:
        for blk in fn.blocks:
            newlist = []
            for inst in blk.instructions:
                si = getattr(inst, "sync_info", None)
                if si is not None and len(si.on_wait) > maxw:
                    waits = list(si.on_wait)
                    for i, w in enumerate(waits[maxw:]):
                        nop = mybir.InstNoOp(
                            name=f"{inst.name}-wsplit{i}",
                            sync_info=mybir.SyncInfo(on_wait=[w], on_update=[]),
                            bass_nofuse=True,
                            engine=inst.engine,
                        )
                        nc.register_instruction(nop)
                        newlist.append(nop)
                    si.on_wait = waits[:maxw]
                newlist.append(inst)
            blk.instructions[:] = newlist
# ---------------------------------------------------------------------------

_NC_CACHE = {}


def build_program():
    if "nc" in _NC_CACHE:
        return _NC_CACHE["nc"]
    f32 = mybir.dt.float32
    bf16 = mybir.dt.bfloat16

    nc = bass.Bass()
    lhsX_d = nc.declare_dram_parameter("lhsX", [KCH, 128, MLOC], bf16, isOutput=False)
    rhsX_d = nc.declare_dram_parameter("rhsX", [KCH, 128, BS], bf16, isOutput=False)
    rhsA_d = nc.declare_dram_parameter("rhsA", [2, BS], bf16, isOutput=False)
    sqb_d = nc.declare_dram_parameter("sqb", [MCH, 128, 1], f32, isOutput=False)
    uu_d = nc.declare_dram_parameter("uu", [MCH, 128, 24], bf16, isOutput=False)
    out_d = nc.declare_dram_parameter("out", [44, BS], f32, isOutput=True)

    with tile.TileContext(nc) as tc, ExitStack() as ctx:
        singles = ctx.enter_context(tc.tile_pool(name="singles", bufs=1))
        work = ctx.enter_context(tc.tile_pool(name="work", bufs=3))
        pd2 = ctx.enter_context(tc.tile_pool(name="pd2", bufs=2, space="PSUM"))
        pT = ctx.enter_context(tc.tile_pool(name="pT", bufs=1, space="PSUM"))

        # small tensors first so compute can start as soon as the first
        # j-slab of BX lands; BX is DMA'd in (jc, k) slabs in consumption
        # order (subtile deps let jc=0 matmuls start after 1/4 of BX).
        ones = singles.tile([2, 128], bf16)
        nc.vector.memset(ones, 1.0)
        sqb = singles.tile([128, MCH, 1], f32)
        for m in range(MCH):
            nc.sync.dma_start(out=sqb[:, m, :], in_=sqb_d[m])
        uu = singles.tile([128, MCH, 24], bf16)
        for m in range(MCH):
            nc.sync.dma_start(out=uu[:, m, :], in_=uu_d[m])
        BA = singles.tile([2, BS], bf16)
        nc.sync.dma_start(out=BA, in_=rhsA_d[:, :])
        AX = singles.tile([128, KCH, MLOC], bf16)
        for k in range(KCH):
            nc.sync.dma_start(out=AX[:, k, :], in_=lhsX_d[k])
        BX = singles.tile([128, KCH, BS], bf16)
        for jc in range(JC):
            jsl = slice(jc * JW, (jc + 1) * JW)
            for k in range(KCH):
                nc.sync.dma_start(out=BX[:, k, jsl], in_=rhsX_d[k, :, jsl])
        Tout = singles.tile([44, BS], f32)

        for jc in range(JC):
            Tsa = pT.tile([12, JW], mybir.dt.float32)
            Ts = pT.tile([12, JW], mybir.dt.float32)
            for m in range(MCH):
                d2 = pd2.tile([128, JW], mybir.dt.float32)
                for h in range(2):
                    n0 = jc * JW + h * 512
                    for k in range(KCH):
                        nc.tensor.matmul(
                            d2[:, h * 512:(h + 1) * 512],
                            AX[:, k, m * 128:(m + 1) * 128],
                            BX[:, k, n0:n0 + 512],
                            start=(k == 0),
                            stop=False,
                        )
                    nc.tensor.matmul(
                        d2[:, h * 512:(h + 1) * 512],
                        ones[:, 0:128],
                        BA[:, n0:n0 + 512],
                        start=False,
                        stop=True,
                    )
                dist = work.tile([128, JW], mybir.dt.bfloat16)
                nc.scalar.activation(
                    out=dist, in_=d2,
                    func=mybir.ActivationFunctionType.Sqrt,
                    bias=sqb[:, m, :], scale=1.0,
                )
                dmin = work.tile([128, JW], mybir.dt.bfloat16)
                nc.vector.tensor_scalar_min(dmin, dist, 1.0)
                for h in range(2):
                    sl = slice(h * 512, (h + 1) * 512)
                    nc.tensor.matmul(
                        Tsa[:, sl], uu[:, m, 0:12], dist[:, sl],
                        start=(m == 0), stop=(m == MCH - 1),
                    )
                    nc.tensor.matmul(
                        Ts[:, sl], uu[:, m, 12:24], dmin[:, sl],
                        start=(m == 0), stop=(m == MCH - 1),
                    )
            nc.scalar.copy(out=Tout[0:12, jc * JW:(jc + 1) * JW], in_=Tsa)
            nc.vector.tensor_copy(out=Tout[32:44, jc * JW:(jc + 1) * JW], in_=Ts)
        nc.sync.dma_start(out=out_d[:, :], in_=Tout)

    _split_waits(nc)
    _NC_CACHE["nc"] = nc
    return nc


def prepare_inputs(X, ds, y):
    X = np.asarray(X, dtype=np.float32)
    ds = np.asarray(ds).astype(np.int64)
    y = np.asarray(y).astype(np.int64)

    Xb16 = X.astype(BF16)
    Xb = Xb16.astype(np.float64)
    sq = (Xb * Xb).sum(axis=1)                      # exact-ish ||xb||^2
    sq32 = sq.astype(np.float32)
    sq_hi = sq32.astype(BF16)
    sq_lo = (sq32 - sq_hi.astype(np.float32)).astype(BF16)

    # rhs: [X^T ; sq_hi ; sq_lo]   (shared by all cores)
    rhsX = np.ascontiguousarray(
        Xb16.T.reshape(KCH, 128, BS))                # (4,128,4096)
    rhsA = np.stack([sq_hi, sq_lo]).astype(BF16)     # (2,4096)

    # masks, rank-12:  r = c*3 + a
    cc = (np.arange(12) // 3)[None, :]               # class of combo r
    aa = (np.arange(12) % 3)[None, :]                # domain of combo r
    U_sa = ((y[:, None] == cc) & (ds[:, None] < aa)).astype(BF16)
    U_s = ((y[:, None] < cc) & (ds[:, None] < aa)).astype(BF16)
    UU = np.concatenate([U_sa, U_s], axis=1)         # (4096, 24)

    in_maps = []
    for c in range(NCORES):
        r0 = c * MLOC
        Xl = Xb16[r0:r0 + MLOC]                      # (512, 512) bf16
        lhsX = np.ascontiguousarray(
            (-2.0 * Xl.astype(np.float32)).astype(BF16).T.reshape(KCH, 128, MLOC))
        sqb = (sq32[r0:r0 + MLOC] + np.float32(C0)).reshape(MCH, 128, 1)
        uu = np.ascontiguousarray(UU[r0:r0 + MLOC].reshape(MCH, 128, 24))
        in_maps.append({
            "lhsX": lhsX,
            "rhsX": rhsX,
            "rhsA": rhsA,
            "sqb": sqb.astype(np.float32),
            "uu": uu,
        })
    return in_maps


def finish(results, ds, y, n_classes, n_domains):
    ds = np.asarray(ds).astype(np.int64)
    y = np.asarray(y).astype(np.int64)
    n_classes = int(n_classes)
    n_domains = int(n_domains)
    combo = (y * 3 + ds).astype(np.int64)
    jj = np.arange(BS)

    sa_sum = 0.0
    smin_sum = 0.0
    for c in range(NCORES):
        T = np.asarray(results[c]["out"], dtype=np.float64)   # (44, 4096)
        sa_sum += T[0:12][combo, jj].sum()
        smin_sum += T[32:44][combo, jj].sum()

    # exact pair count for the s mask
    cnt = np.bincount(combo, minlength=12).astype(np.float64)
    cc = np.arange(12) // 3
    aa = np.arange(12) % 3
    Ms = ((cc[:, None] < cc[None, :]) & (aa[:, None] < aa[None, :])).astype(np.float64)
    n_pairs_s = cnt @ Ms @ cnt

    n_sa = n_classes * (n_domains * (n_domains - 1) // 2)
    n_s = (n_classes * (n_classes - 1) // 2) * (n_domains * (n_domains - 1) // 2)
    sa_loss = 0.5 * sa_sum / n_sa
    s_loss = 0.5 * (n_pairs_s - smin_sum) / n_s
    return np.array([sa_loss, s_loss], dtype=np.float32)


def run_device(in_maps, trace=False, **kw):
    nc = build_program()
    return run_bass_kernel_spmd(nc, in_maps, core_ids=list(range(NCORES)),
                                trace=trace, **kw)


def kernel(X, ds, y, n_classes, n_domains):
    in_maps = prepare_inputs(X, ds, y)
    res = run_device(in_maps)
    return finish(res.results, ds, y, n_classes, n_domains)


# revision 7
# speedup vs baseline: 1.0695x; 1.0695x over previous
"""JointCCSA loss kernel for 8 Trainium2 NeuronCores.

reference:
    dists = cdist(X, X)                                  (bs, bs)
    sa_loss = 0.5 * sum[ same_y & ds_lt ] dists / n_sa
    s_loss  = 0.5 * sum[ y_lt  & ds_lt ] relu(1 - dists) / n_s

Strategy (data-parallel over rows of X, 8 cores, 512 rows each):
  * d2(i,j) = ||xb_i - xb_j||^2 via ONE augmented bf16 matmul into PSUM:
      lhsT = [-2*Xb_loc ; 1 ; 1]  (K=514),  rhs = [Xb^T ; sq_hi ; sq_lo]
    with sq = sum(bf16(X)^2) (so the Gram matrix is the exact-squared-dist
    of the rounded points -> d2 >= -eps, no NaN from sqrt).
  * dist = Sqrt(d2 + (sq_i + c0)) on ScalarE straight from PSUM (bias is
    per-partition), c0 = 0.0625 guards fp32-accumulation noise on the diag.
  * The pair masks are rank-12: mask(i,j) = e_i^T M e_j with e = onehot of
    (y, ds) combo (4*3=12).  So the masked reductions become tiny matmuls:
      T_sa(r,j) = sum_i U_sa(i,r) * dist(i,j)      U_sa(i,(c,a)) = [y_i==c][ds_i<a]
      T_s (r,j) = sum_i U_s (i,r) * min(dist,1)    U_s (i,(c,a)) = [y_i<c][ds_i<a]
    (min(d,1) = 1 - relu(1-d), so  sum A_s*relu(1-d) = N_pairs - sum A_s*min(d,1))
  * Host gathers T[combo(j), j] (one-hot contraction -> exact diag exclusion)
    and sums across cores.  Output: np.array([sa_loss, s_loss], float32).
"""

import numpy as np
import ml_dtypes
from contextlib import ExitStack

import concourse.bass as bass
import concourse.tile as tile
from concourse import mybir
from concourse.vector_clock import ScopedClock
from concourse.bass_utils import run_bass_kernel_spmd

BS = 4096
D = 512
NCORES = 8
MLOC = BS // NCORES          # 512 rows per core
MCH = MLOC // 128            # 4 partition chunks of local rows
KCH = D // 128               # 4 contraction chunks of X dims
JC = 4                       # j-chunks of width 1024
JW = 1024
C0 = 0.0625                  # sqrt-safety bias added into sq_i
BF16 = ml_dtypes.bfloat16


# ---------------------------------------------------------------------------
# Patch: this walrus build allows only ONE sync-wait on a CTRL-type (Drain)
# instruction; Tile's final drain aggregates many.  Spread them over
# single-wait SP nops.
def _patched_drain_and_barrier(self, tick_clock, wait_clock):
    nc = self.nc
    coll = nc.sync.nop(nofuse=True, hint="drain_wait_collector")
    wait_clock.add_sem_waits(coll.ins, ScopedClock({None: tick_clock.global_clock}))
    si = coll.ins.sync_info
    waits = list(si.on_wait) if si is not None else []
    if len(waits) > 1:
        si.on_wait = [waits[0]]
        for w in waits[1:]:
            n = nc.sync.nop(nofuse=True, hint="drain_wait_extra")
            n.ins.sync_info = mybir.SyncInfo(on_wait=[w], on_update=[])
    nc.sync.drain()
    nc.all_engine_barrier()
    assert self.sems is not None
    popped = nc._tile_sem_poison_stack.pop()
    assert popped is self._sem_poison
    nc.clear_and_free_semaphores(list(self.sems.allocated().values()))
    nc.all_engine_barrier()


tile.TileContext._drain_and_barrier = _patched_drain_and_barrier


def _split_waits(nc, maxw=1):
    """Hoist extra sync-waits from every instruction onto same-engine NoOps
    (this walrus build rejects instructions with more than ~1 wait)."""
    for fn in nc.m.functions:
        for blk in fn.blocks:
            newlist = []
            for inst in blk.instructions:
                si = getattr(inst, "sync_info", None)
                if si is not None and len(si.on_wait) > maxw:
                    waits = list(si.on_wait)
                    for i, w in enumerate(waits[maxw:]):
                        nop = mybir.InstNoOp(
                            name=f"{inst.name}-wsplit{i}",
                            sync_info=mybir.SyncInfo(on_wait=[w], on_update=[]),
                            bass_nofuse=True,
                            engine=inst.engine,
                        )
                        nc.register_instruction(nop)
                        newlist.append(nop)
                    si.on_wait = waits[:maxw]
                newlist.append(inst)
            blk.instructions[:] = newlist
# ---------------------------------------------------------------------------

_NC_CACHE = {}


def build_program():
    if "nc" in _NC_CACHE:
        return _NC_CACHE["nc"]
    f32 = mybir.dt.float32
    bf16 = mybir.dt.bfloat16

    nc = bass.Bass()
    lhsX_d = nc.declare_dram_parameter("lhsX", [KCH, 128, MLOC], bf16, isOutput=False)
    rhsX_d = nc.declare_dram_parameter("rhsX", [KCH, 128, BS], bf16, isOutput=False)
    rhsA_d = nc.declare_dram_parameter("rhsA", [2, BS], bf16, isOutput=False)
    sqb_d = nc.declare_dram_parameter("sqb", [MCH, 128, 1], f32, isOutput=False)
    uu_d = nc.declare_dram_parameter("uu", [MCH, 128, 24], bf16, isOutput=False)
    out_d = nc.declare_dram_parameter("out", [44, BS], f32, isOutput=True)

    with tile.TileContext(nc) as tc, ExitStack() as ctx:
        singles = ctx.enter_context(tc.tile_pool(name="singles", bufs=1))
        work = ctx.enter_context(tc.tile_pool(name="work", bufs=3))
        pd2 = ctx.enter_context(tc.tile_pool(name="pd2", bufs=2, space="PSUM"))
        pT = ctx.enter_context(tc.tile_pool(name="pT", bufs=1, space="PSUM"))

        # Consolidated DMAs (each dma_start costs ~600ns of Sync issue time):
        # small tensors + the first j-slab of BX first so matmuls start
        # early; the remaining 3/4 of BX streams in behind them.
        ones = singles.tile([2, 128], bf16)
        nc.vector.memset(ones, 1.0)
        sqb = singles.tile([128, MCH], f32)
        nc.sync.dma_start(out=sqb, in_=sqb_d[:, :, 0].rearrange("m p -> p m"))
        uu = singles.tile([128, MCH, 24], bf16)
        nc.sync.dma_start(out=uu, in_=uu_d[:, :, :].rearrange("m p u -> p m u"))
        BA = singles.tile([2, BS], bf16)
        nc.sync.dma_start(out=BA, in_=rhsA_d[:, :])
        AX = singles.tile([128, KCH, MLOC], bf16)
        nc.sync.dma_start(out=AX, in_=lhsX_d[:, :, :].rearrange("k p m -> p k m"))
        BX = singles.tile([128, KCH, BS], bf16)
        nc.sync.dma_start(
            out=BX[:, :, 0:JW],
            in_=rhsX_d[:, :, 0:JW].rearrange("k p j -> p k j"))
        nc.sync.dma_start(
            out=BX[:, :, JW:BS],
            in_=rhsX_d[:, :, JW:BS].rearrange("k p j -> p k j"))
        Tout = singles.tile([44, BS], f32)

        for jc in range(JC):
            Tsa = pT.tile([12, JW], mybir.dt.float32)
            Ts = pT.tile([12, JW], mybir.dt.float32)
            for m in range(MCH):
                d2 = pd2.tile([128, JW], mybir.dt.float32)
                for h in range(2):
                    n0 = jc * JW + h * 512
                    for k in range(KCH):
                        nc.tensor.matmul(
                            d2[:, h * 512:(h + 1) * 512],
                            AX[:, k, m * 128:(m + 1) * 128],
                            BX[:, k, n0:n0 + 512],
                            start=(k == 0),
                            stop=False,
                        )
                    nc.tensor.matmul(
                        d2[:, h * 512:(h + 1) * 512],
                        ones[:, 0:128],
                        BA[:, n0:n0 + 512],
                        start=False,
                        stop=True,
                    )
                dist = work.tile([128, JW], mybir.dt.bfloat16)
                nc.scalar.activation(
                    out=dist, in_=d2,
                    func=mybir.ActivationFunctionType.Sqrt,
                    bias=sqb[:, m:m + 1], scale=1.0,
                )
                dmin = work.tile([128, JW], mybir.dt.bfloat16)
                nc.vector.tensor_scalar_min(dmin, dist, 1.0)
                for h in range(2):
                    sl = slice(h * 512, (h + 1) * 512)
                    nc.tensor.matmul(
                        Tsa[:, sl], uu[:, m, 0:12], dist[:, sl],
                        start=(m == 0), stop=(m == MCH - 1),
                    )
                    nc.tensor.matmul(
                        Ts[:, sl], uu[:, m, 12:24], dmin[:, sl],
                        start=(m == 0), stop=(m == MCH - 1),
                    )
            nc.scalar.copy(out=Tout[0:12, jc * JW:(jc + 1) * JW], in_=Tsa)
            nc.vector.tensor_copy(out=Tout[32:44, jc * JW:(jc + 1) * JW], in_=Ts)
        nc.sync.dma_start(out=out_d[:, :], in_=Tout)

    _split_waits(nc)
    _NC_CACHE["nc"] = nc
    return nc


def prepare_inputs(X, ds, y):
    X = np.asarray(X, dtype=np.float32)
    ds = np.asarray(ds).astype(np.int64)
    y = np.asarray(y).astype(np.int64)

    Xb16 = X.astype(BF16)
    Xb = Xb16.astype(np.float64)
    sq = (Xb * Xb).sum(axis=1)                      # exact-ish ||xb||^2
    sq32 = sq.astype(np.float32)
    sq_hi = sq32.astype(BF16)
    sq_lo = (sq32 - sq_hi.astype(np.float32)).astype(BF16)

    # rhs: [X^T ; sq_hi ; sq_lo]   (shared by all cores)
    rhsX = np.ascontiguousarray(
        Xb16.T.reshape(KCH, 128, BS))                # (4,128,4096)
    rhsA = np.stack([sq_hi, sq_lo]).astype(BF16)     # (2,4096)

    # masks, rank-12:  r = c*3 + a
    cc = (np.arange(12) // 3)[None, :]               # class of combo r
    aa = (np.arange(12) % 3)[None, :]                # domain of combo r
    U_sa = ((y[:, None] == cc) & (ds[:, None] < aa)).astype(BF16)
    U_s = ((y[:, None] < cc) & (ds[:, None] < aa)).astype(BF16)
    UU = np.concatenate([U_sa, U_s], axis=1)         # (4096, 24)

    in_maps = []
    for c in range(NCORES):
        r0 = c * MLOC
        Xl = Xb16[r0:r0 + MLOC]                      # (512, 512) bf16
        lhsX = np.ascontiguousarray(
            (-2.0 * Xl.astype(np.float32)).astype(BF16).T.reshape(KCH, 128, MLOC))
        sqb = (sq32[r0:r0 + MLOC] + np.float32(C0)).reshape(MCH, 128, 1)
        uu = np.ascontiguousarray(UU[r0:r0 + MLOC].reshape(MCH, 128, 24))
        in_maps.append({
            "lhsX": lhsX,
            "rhsX": rhsX,
            "rhsA": rhsA,
            "sqb": sqb.astype(np.float32),
            "uu": uu,
        })
    return in_maps


def finish(results, ds, y, n_classes, n_domains):
    ds = np.asarray(ds).astype(np.int64)
    y = np.asarray(y).astype(np.int64)
    n_classes = int(n_classes)
    n_domains = int(n_domains)
    combo = (y * 3 + ds).astype(np.int64)
    jj = np.arange(BS)

    sa_sum = 0.0
    smin_sum = 0.0
    for c in range(NCORES):
        T = np.asarray(results[c]["out"], dtype=np.float64)   # (44, 4096)
        sa_sum += T[0:12][combo, jj].sum()
        smin_sum += T[32:44][combo, jj].sum()

    # exact pair count for the s mask
    cnt = np.bincount(combo, minlength=12).astype(np.float64)
    cc = np.arange(12) // 3
    aa = np.arange(12) % 3
    Ms = ((cc[:, None] < cc[None, :]) & (aa[:, None] < aa[None, :])).astype(np.float64)
    n_pairs_s = cnt @ Ms @ cnt

    n_sa = n_classes * (n_domains * (n_domains - 1) // 2)
    n_s = (n_classes * (n_classes - 1) // 2) * (n_domains * (n_domains - 1) // 2)
    sa_loss = 0.5 * sa_sum / n_sa
    s_loss = 0.5 * (n_pairs_s - smin_sum) / n_s
    return np.array([sa_loss, s_loss], dtype=np.float32)


def run_device(in_maps, trace=False, **kw):
    nc = build_program()
    return run_bass_kernel_spmd(nc, in_maps, core_ids=list(range(NCORES)),
                                trace=trace, **kw)


def kernel(X, ds, y, n_classes, n_domains):
    in_maps = prepare_inputs(X, ds, y)
    res = run_device(in_maps)
    return finish(res.results, ds, y, n_classes, n_domains)


# revision 9
# speedup vs baseline: 1.0988x; 1.0274x over previous
"""JointCCSA loss kernel for 8 Trainium2 NeuronCores.

reference:
    dists = cdist(X, X)                                  (bs, bs)
    sa_loss = 0.5 * sum[ same_y & ds_lt ] dists / n_sa
    s_loss  = 0.5 * sum[ y_lt  & ds_lt ] relu(1 - dists) / n_s

Strategy (data-parallel over rows of X, 8 cores, 512 rows each):
  * d2(i,j) = ||xb_i - xb_j||^2 via ONE augmented bf16 matmul into PSUM:
      lhsT = [-2*Xb_loc ; 1 ; 1]  (K=514),  rhs = [Xb^T ; sq_hi ; sq_lo]
    with sq = sum(bf16(X)^2) (so the Gram matrix is the exact-squared-dist
    of the rounded points -> d2 >= -eps, no NaN from sqrt).
  * dist = Sqrt(d2 + (sq_i + c0)) on ScalarE straight from PSUM (bias is
    per-partition), c0 = 0.0625 guards fp32-accumulation noise on the diag.
  * The pair masks are rank-12: mask(i,j) = e_i^T M e_j with e = onehot of
    (y, ds) combo (4*3=12).  So the masked reductions become tiny matmuls:
      T_sa(r,j) = sum_i U_sa(i,r) * dist(i,j)      U_sa(i,(c,a)) = [y_i==c][ds_i<a]
      T_s (r,j) = sum_i U_s (i,r) * min(dist,1)    U_s (i,(c,a)) = [y_i<c][ds_i<a]
    (min(d,1) = 1 - relu(1-d), so  sum A_s*relu(1-d) = N_pairs - sum A_s*min(d,1))
  * Host gathers T[combo(j), j] (one-hot contraction -> exact diag exclusion)
    and sums across cores.  Output: np.array([sa_loss, s_loss], float32).
"""

import numpy as np
import ml_dtypes
from contextlib import ExitStack

import concourse.bass as bass
import concourse.tile as tile
from concourse import mybir
from concourse.vector_clock import ScopedClock
from concourse.bass_utils import run_bass_kernel_spmd

BS = 4096
D = 512
NCORES = 8
MLOC = BS // NCORES          # 512 rows per core
MCH = MLOC // 128            # 4 partition chunks of local rows
KCH = D // 128               # 4 contraction chunks of X dims
JC = 4                       # j-chunks of width 1024
JW = 1024
C0 = 0.0625                  # sqrt-safety bias added into sq_i
BF16 = ml_dtypes.bfloat16


# ---------------------------------------------------------------------------
# Patch: this walrus build allows only ONE sync-wait on a CTRL-type (Drain)
# instruction; Tile's final drain aggregates many.  Spread them over
# single-wait SP nops.
def _patched_drain_and_barrier(self, tick_clock, wait_clock):
    nc = self.nc
    coll = nc.sync.nop(nofuse=True, hint="drain_wait_collector")
    wait_clock.add_sem_waits(coll.ins, ScopedClock({None: tick_clock.global_clock}))
    si = coll.ins.sync_info
    waits = list(si.on_wait) if si is not None else []
    if len(waits) > 1:
        si.on_wait = [waits[0]]
        for w in waits[1:]:
            n = nc.sync.nop(nofuse=True, hint="drain_wait_extra")
            n.ins.sync_info = mybir.SyncInfo(on_wait=[w], on_update=[])
    nc.sync.drain()
    nc.all_engine_barrier()
    assert self.sems is not None
    popped = nc._tile_sem_poison_stack.pop()
    assert popped is self._sem_poison
    nc.clear_and_free_semaphores(list(self.sems.allocated().values()))
    nc.all_engine_barrier()


tile.TileContext._drain_and_barrier = _patched_drain_and_barrier


def _split_waits(nc, maxw=1):
    """Hoist extra sync-waits from every instruction onto same-engine NoOps
    (this walrus build rejects instructions with more than ~1 wait)."""
    for fn in nc.m.functions:
        for blk in fn.blocks:
            newlist = []
            for inst in blk.instructions:
                si = getattr(inst, "sync_info", None)
                if si is not None and len(si.on_wait) > maxw:
                    waits = list(si.on_wait)
                    for i, w in enumerate(waits[maxw:]):
                        nop = mybir.InstNoOp(
                            name=f"{inst.name}-wsplit{i}",
                            sync_info=mybir.SyncInfo(on_wait=[w], on_update=[]),
                            bass_nofuse=True,
                            engine=inst.engine,
                        )
                        nc.register_instruction(nop)
                        newlist.append(nop)
                    si.on_wait = waits[:maxw]
                newlist.append(inst)
            blk.instructions[:] = newlist
# ---------------------------------------------------------------------------

_NC_CACHE = {}


def build_program():
    if "nc" in _NC_CACHE:
        return _NC_CACHE["nc"]
    f32 = mybir.dt.float32
    bf16 = mybir.dt.bfloat16

    nc = bass.Bass()
    lhsX_d = nc.declare_dram_parameter("lhsX", [KCH, 128, MLOC], bf16, isOutput=False)
    rhsX_d = nc.declare_dram_parameter("rhsX", [KCH, 128, BS], bf16, isOutput=False)
    rhsA_d = nc.declare_dram_parameter("rhsA", [2, BS], bf16, isOutput=False)
    sqb_d = nc.declare_dram_parameter("sqb", [MCH, 128, 1], f32, isOutput=False)
    uu_d = nc.declare_dram_parameter("uu", [MCH, 128, 24], bf16, isOutput=False)
    out_d = nc.declare_dram_parameter("out", [44, BS], f32, isOutput=True)

    with tile.TileContext(nc) as tc, ExitStack() as ctx:
        singles = ctx.enter_context(tc.tile_pool(name="singles", bufs=1))
        work = ctx.enter_context(tc.tile_pool(name="work", bufs=3))
        pd2 = ctx.enter_context(tc.tile_pool(name="pd2", bufs=2, space="PSUM"))
        pT = ctx.enter_context(tc.tile_pool(name="pT", bufs=1, space="PSUM"))

        # Consolidated DMAs (each dma_start costs ~600ns of Sync issue time):
        # small tensors + the first j-slab of BX first so matmuls start
        # early; the remaining 3/4 of BX streams in behind them.
        ones = singles.tile([2, 128], bf16)
        nc.vector.memset(ones, 1.0)
        sqb = singles.tile([128, MCH], f32)
        nc.gpsimd.dma_start(out=sqb, in_=sqb_d[:, :, 0].rearrange("m p -> p m"))
        uu = singles.tile([128, MCH, 24], bf16)
        nc.gpsimd.dma_start(out=uu, in_=uu_d[:, :, :].rearrange("m p u -> p m u"))
        BA = singles.tile([2, BS], bf16)
        nc.gpsimd.dma_start(out=BA, in_=rhsA_d[:, :])
        AX = singles.tile([128, KCH, MLOC], bf16)
        nc.scalar.dma_start(out=AX, in_=lhsX_d[:, :, :].rearrange("k p m -> p k m"))
        BX = singles.tile([128, KCH, BS], bf16)
        nc.sync.dma_start(
            out=BX[:, :, 0:JW],
            in_=rhsX_d[:, :, 0:JW].rearrange("k p j -> p k j"))
        nc.gpsimd.dma_start(
            out=BX[:, :, JW:BS],
            in_=rhsX_d[:, :, JW:BS].rearrange("k p j -> p k j"))
        Tout = singles.tile([44, BS], f32)

        for jc in range(JC):
            Tsa = pT.tile([12, JW], mybir.dt.float32)
            Ts = pT.tile([12, JW], mybir.dt.float32)
            for m in range(MCH):
                d2 = pd2.tile([128, JW], mybir.dt.float32)
                for h in range(2):
                    n0 = jc * JW + h * 512
                    for k in range(KCH):
                        nc.tensor.matmul(
                            d2[:, h * 512:(h + 1) * 512],
                            AX[:, k, m * 128:(m + 1) * 128],
                            BX[:, k, n0:n0 + 512],
                            start=(k == 0),
                            stop=False,
                        )
                    nc.tensor.matmul(
                        d2[:, h * 512:(h + 1) * 512],
                        ones[:, 0:128],
                        BA[:, n0:n0 + 512],
                        start=False,
                        stop=True,
                    )
                dist = work.tile([128, JW], mybir.dt.bfloat16)
                nc.scalar.activation(
                    out=dist, in_=d2,
                    func=mybir.ActivationFunctionType.Sqrt,
                    bias=sqb[:, m:m + 1], scale=1.0,
                )
                dmin = work.tile([128, JW], mybir.dt.bfloat16)
                nc.vector.tensor_scalar_min(dmin, dist, 1.0)
                for h in range(2):
                    sl = slice(h * 512, (h + 1) * 512)
                    nc.tensor.matmul(
                        Tsa[:, sl], uu[:, m, 0:12], dist[:, sl],
                        start=(m == 0), stop=(m == MCH - 1),
                    )
                    nc.tensor.matmul(
                        Ts[:, sl], uu[:, m, 12:24], dmin[:, sl],
                        start=(m == 0), stop=(m == MCH - 1),
                    )
            nc.scalar.copy(out=Tout[0:12, jc * JW:(jc + 1) * JW], in_=Tsa)
            nc.vector.tensor_copy(out=Tout[32:44, jc * JW:(jc + 1) * JW], in_=Ts)
        nc.sync.dma_start(out=out_d[:, :], in_=Tout)

    _split_waits(nc)
    _NC_CACHE["nc"] = nc
    return nc


def prepare_inputs(X, ds, y):
    X = np.asarray(X, dtype=np.float32)
    ds = np.asarray(ds).astype(np.int64)
    y = np.asarray(y).astype(np.int64)

    Xb16 = X.astype(BF16)
    Xb = Xb16.astype(np.float64)
    sq = (Xb * Xb).sum(axis=1)                      # exact-ish ||xb||^2
    sq32 = sq.astype(np.float32)
    sq_hi = sq32.astype(BF16)
    sq_lo = (sq32 - sq_hi.astype(np.float32)).astype(BF16)

    # rhs: [X^T ; sq_hi ; sq_lo]   (shared by all cores)
    rhsX = np.ascontiguousarray(
        Xb16.T.reshape(KCH, 128, BS))                # (4,128,4096)
    rhsA = np.stack([sq_hi, sq_lo]).astype(BF16)     # (2,4096)

    # masks, rank-12:  r = c*3 + a
    cc = (np.arange(12) // 3)[None, :]               # class of combo r
    aa = (np.arange(12) % 3)[None, :]                # domain of combo r
    U_sa = ((y[:, None] == cc) & (ds[:, None] < aa)).astype(BF16)
    U_s = ((y[:, None] < cc) & (ds[:, None] < aa)).astype(BF16)
    UU = np.concatenate([U_sa, U_s], axis=1)         # (4096, 24)

    in_maps = []
    for c in range(NCORES):
        r0 = c * MLOC
        Xl = Xb16[r0:r0 + MLOC]                      # (512, 512) bf16
        lhsX = np.ascontiguousarray(
            (-2.0 * Xl.astype(np.float32)).astype(BF16).T.reshape(KCH, 128, MLOC))
        sqb = (sq32[r0:r0 + MLOC] + np.float32(C0)).reshape(MCH, 128, 1)
        uu = np.ascontiguousarray(UU[r0:r0 + MLOC].reshape(MCH, 128, 24))
        in_maps.append({
            "lhsX": lhsX,
            "rhsX": rhsX,
            "rhsA": rhsA,
            "sqb": sqb.astype(np.float32),
            "uu": uu,
        })
    return in_maps


def finish(results, ds, y, n_classes, n_domains):
    ds = np.asarray(ds).astype(np.int64)
    y = np.asarray(y).astype(np.int64)
    n_classes = int(n_classes)
    n_domains = int(n_domains)
    combo = (y * 3 + ds).astype(np.int64)
    jj = np.arange(BS)

    sa_sum = 0.0
    smin_sum = 0.0
    for c in range(NCORES):
        T = np.asarray(results[c]["out"], dtype=np.float64)   # (44, 4096)
        sa_sum += T[0:12][combo, jj].sum()
        smin_sum += T[32:44][combo, jj].sum()

    # exact pair count for the s mask
    cnt = np.bincount(combo, minlength=12).astype(np.float64)
    cc = np.arange(12) // 3
    aa = np.arange(12) % 3
    Ms = ((cc[:, None] < cc[None, :]) & (aa[:, None] < aa[None, :])).astype(np.float64)
    n_pairs_s = cnt @ Ms @ cnt

    n_sa = n_classes * (n_domains * (n_domains - 1) // 2)
    n_s = (n_classes * (n_classes - 1) // 2) * (n_domains * (n_domains - 1) // 2)
    sa_loss = 0.5 * sa_sum / n_sa
    s_loss = 0.5 * (n_pairs_s - smin_sum) / n_s
    return np.array([sa_loss, s_loss], dtype=np.float32)


def run_device(in_maps, trace=False, **kw):
    nc = build_program()
    return run_bass_kernel_spmd(nc, in_maps, core_ids=list(range(NCORES)),
                                trace=trace, **kw)


def kernel(X, ds, y, n_classes, n_domains):
    in_maps = prepare_inputs(X, ds, y)
    res = run_device(in_maps)
    return finish(res.results, ds, y, n_classes, n_domains)


# revision 10
# speedup vs baseline: 1.1874x; 1.0806x over previous
"""JointCCSA loss kernel for 8 Trainium2 NeuronCores.

reference:
    dists = cdist(X, X)                                  (bs, bs)
    sa_loss = 0.5 * sum[ same_y & ds_lt ] dists / n_sa
    s_loss  = 0.5 * sum[ y_lt  & ds_lt ] relu(1 - dists) / n_s

Strategy (data-parallel over rows of X, 8 cores, 512 rows each):
  * d2(i,j) = ||xb_i - xb_j||^2 via ONE augmented bf16 matmul into PSUM:
      lhsT = [-2*Xb_loc ; 1 ; 1]  (K=514),  rhs = [Xb^T ; sq_hi ; sq_lo]
    with sq = sum(bf16(X)^2) (so the Gram matrix is the exact-squared-dist
    of the rounded points -> d2 >= -eps, no NaN from sqrt).
  * dist = Sqrt(d2 + (sq_i + c0)) on ScalarE straight from PSUM (bias is
    per-partition), c0 = 0.0625 guards fp32-accumulation noise on the diag.
  * The pair masks are rank-12: mask(i,j) = e_i^T M e_j with e = onehot of
    (y, ds) combo (4*3=12).  So the masked reductions become tiny matmuls:
      T_sa(r,j) = sum_i U_sa(i,r) * dist(i,j)      U_sa(i,(c,a)) = [y_i==c][ds_i<a]
      T_s (r,j) = sum_i U_s (i,r) * min(dist,1)    U_s (i,(c,a)) = [y_i<c][ds_i<a]
    (min(d,1) = 1 - relu(1-d), so  sum A_s*relu(1-d) = N_pairs - sum A_s*min(d,1))
  * Host gathers T[combo(j), j] (one-hot contraction -> exact diag exclusion)
    and sums across cores.  Output: np.array([sa_loss, s_loss], float32).
"""

import numpy as np
import ml_dtypes
from contextlib import ExitStack

import concourse.bass as bass
import concourse.tile as tile
from concourse import mybir
from concourse.vector_clock import ScopedClock
from concourse.bass_utils import run_bass_kernel_spmd

BS = 4096
D = 512
NCORES = 8
MLOC = BS // NCORES          # 512 rows per core
MCH = MLOC // 128            # 4 partition chunks of local rows
KCH = D // 128               # 4 contraction chunks of X dims
JC = 4                       # j-chunks of width 1024
JW = 1024
C0 = 0.0625                  # sqrt-safety bias added into sq_i
BF16 = ml_dtypes.bfloat16


# ---------------------------------------------------------------------------
# Patch: this walrus build allows only ONE sync-wait on a CTRL-type (Drain)
# instruction; Tile's final drain aggregates many.  Spread them over
# single-wait SP nops.
def _patched_drain_and_barrier(self, tick_clock, wait_clock):
    nc = self.nc
    coll = nc.sync.nop(nofuse=True, hint="drain_wait_collector")
    wait_clock.add_sem_waits(coll.ins, ScopedClock({None: tick_clock.global_clock}))
    si = coll.ins.sync_info
    waits = list(si.on_wait) if si is not None else []
    if len(waits) > 1:
        si.on_wait = [waits[0]]
        for w in waits[1:]:
            n = nc.sync.nop(nofuse=True, hint="drain_wait_extra")
            n.ins.sync_info = mybir.SyncInfo(on_wait=[w], on_update=[])
    nc.sync.drain()
    nc.all_engine_barrier()
    assert self.sems is not None
    popped = nc._tile_sem_poison_stack.pop()
    assert popped is self._sem_poison
    nc.clear_and_free_semaphores(list(self.sems.allocated().values()))
    nc.all_engine_barrier()


tile.TileContext._drain_and_barrier = _patched_drain_and_barrier


def _split_waits(nc, maxw=1):
    """Hoist extra sync-waits from every instruction onto same-engine NoOps
    (this walrus build rejects instructions with more than ~1 wait)."""
    for fn in nc.m.functions:
        for blk in fn.blocks:
            newlist = []
            for inst in blk.instructions:
                si = getattr(inst, "sync_info", None)
                if si is not None and len(si.on_wait) > maxw:
                    waits = list(si.on_wait)
                    for i, w in enumerate(waits[maxw:]):
                        nop = mybir.InstNoOp(
                            name=f"{inst.name}-wsplit{i}",
                            sync_info=mybir.SyncInfo(on_wait=[w], on_update=[]),
                            bass_nofuse=True,
                            engine=inst.engine,
                        )
                        nc.register_instruction(nop)
                        newlist.append(nop)
                    si.on_wait = waits[:maxw]
                newlist.append(inst)
            blk.instructions[:] = newlist
# ---------------------------------------------------------------------------

_NC_CACHE = {}


def build_program():
    if "nc" in _NC_CACHE:
        return _NC_CACHE["nc"]
    f32 = mybir.dt.float32
    bf16 = mybir.dt.bfloat16

    nc = bass.Bass()
    lhsX_d = nc.declare_dram_parameter("lhsX", [KCH, 128, MLOC], bf16, isOutput=False)
    rhsX_d = nc.declare_dram_parameter("rhsX", [KCH, 128, BS], bf16, isOutput=False)
    sqj_d = nc.declare_dram_parameter("sqj", [1, BS], f32, isOutput=False)
    sqb_d = nc.declare_dram_parameter("sqb", [MCH, 128, 1], f32, isOutput=False)
    uu_d = nc.declare_dram_parameter("uu", [MCH, 128, 24], bf16, isOutput=False)
    out_d = nc.declare_dram_parameter("out", [44, BS], f32, isOutput=True)

    with tile.TileContext(nc) as tc, ExitStack() as ctx:
        singles = ctx.enter_context(tc.tile_pool(name="singles", bufs=1))
        work = ctx.enter_context(tc.tile_pool(name="work", bufs=3))
        pd2 = ctx.enter_context(tc.tile_pool(name="pd2", bufs=2, space="PSUM"))
        pT = ctx.enter_context(tc.tile_pool(name="pT", bufs=1, space="PSUM"))

        # Consolidated DMAs (each dma_start costs ~600ns of Sync issue time):
        # small tensors + the first j-slab of BX first so matmuls start
        # early; the remaining 3/4 of BX streams in behind them.
        sqb = singles.tile([128, MCH], f32)
        nc.gpsimd.dma_start(out=sqb, in_=sqb_d[:, :, 0].rearrange("m p -> p m"))
        uu = singles.tile([128, MCH, 24], bf16)
        nc.gpsimd.dma_start(out=uu, in_=uu_d[:, :, :].rearrange("m p u -> p m u"))
        sqjb = singles.tile([128, BS], f32)
        nc.gpsimd.dma_start(out=sqjb, in_=bass.AP(
            tensor=sqj_d[0].tensor, offset=0, ap=[[0, 128], [1, BS]]))
        AX = singles.tile([128, KCH, MLOC], bf16)
        nc.scalar.dma_start(out=AX, in_=lhsX_d[:, :, :].rearrange("k p m -> p k m"))
        BX = singles.tile([128, KCH, BS], bf16)
        nc.sync.dma_start(
            out=BX[:, :, 0:JW],
            in_=rhsX_d[:, :, 0:JW].rearrange("k p j -> p k j"))
        nc.gpsimd.dma_start(
            out=BX[:, :, JW:BS],
            in_=rhsX_d[:, :, JW:BS].rearrange("k p j -> p k j"))
        Tout = singles.tile([44, BS], f32)

        for jc in range(JC):
            Tsa = pT.tile([12, JW], mybir.dt.float32)
            Ts = pT.tile([12, JW], mybir.dt.float32)
            for m in range(MCH):
                d2 = pd2.tile([128, JW], mybir.dt.float32)
                for h in range(2):
                    n0 = jc * JW + h * 512
                    for k in range(KCH):
                        nc.tensor.matmul(
                            d2[:, h * 512:(h + 1) * 512],
                            AX[:, k, m * 128:(m + 1) * 128],
                            BX[:, k, n0:n0 + 512],
                            start=(k == 0),
                            stop=(k == KCH - 1),
                        )
                d2c = work.tile([128, JW], mybir.dt.float32)
                nc.vector.tensor_add(
                    d2c, d2, sqjb[:, jc * JW:(jc + 1) * JW])
                dist = work.tile([128, JW], mybir.dt.bfloat16)
                nc.scalar.activation(
                    out=dist, in_=d2c,
                    func=mybir.ActivationFunctionType.Sqrt,
                    bias=sqb[:, m:m + 1], scale=1.0,
                )
                dmin = work.tile([128, JW], mybir.dt.bfloat16)
                nc.vector.tensor_scalar_min(dmin, dist, 1.0)
                for h in range(2):
                    sl = slice(h * 512, (h + 1) * 512)
                    nc.tensor.matmul(
                        Tsa[:, sl], uu[:, m, 0:12], dist[:, sl],
                        start=(m == 0), stop=(m == MCH - 1),
                    )
                    nc.tensor.matmul(
                        Ts[:, sl], uu[:, m, 12:24], dmin[:, sl],
                        start=(m == 0), stop=(m == MCH - 1),
                    )
            nc.scalar.copy(out=Tout[0:12, jc * JW:(jc + 1) * JW], in_=Tsa)
            nc.vector.tensor_copy(out=Tout[32:44, jc * JW:(jc + 1) * JW], in_=Ts)
        nc.sync.dma_start(out=out_d[:, :], in_=Tout)

    _split_waits(nc)
    _NC_CACHE["nc"] = nc
    return nc


def prepare_inputs(X, ds, y):
    X = np.asarray(X, dtype=np.float32)
    ds = np.asarray(ds).astype(np.int64)
    y = np.asarray(y).astype(np.int64)

    Xb16 = X.astype(BF16)
    Xb = Xb16.astype(np.float64)
    sq = (Xb * Xb).sum(axis=1)                      # exact-ish ||xb||^2
    sq32 = sq.astype(np.float32)
    sq_hi = sq32.astype(BF16)
    sq_lo = (sq32 - sq_hi.astype(np.float32)).astype(BF16)

    # rhs: [X^T ; sq_hi ; sq_lo]   (shared by all cores)
    rhsX = np.ascontiguousarray(
        Xb16.T.reshape(KCH, 128, BS))                # (4,128,4096)
    sqj = sq32.reshape(1, BS)                        # (1,4096) f32

    # masks, rank-12:  r = c*3 + a
    cc = (np.arange(12) // 3)[None, :]               # class of combo r
    aa = (np.arange(12) % 3)[None, :]                # domain of combo r
    U_sa = ((y[:, None] == cc) & (ds[:, None] < aa)).astype(BF16)
    U_s = ((y[:, None] < cc) & (ds[:, None] < aa)).astype(BF16)
    UU = np.concatenate([U_sa, U_s], axis=1)         # (4096, 24)

    in_maps = []
    for c in range(NCORES):
        r0 = c * MLOC
        Xl = Xb16[r0:r0 + MLOC]                      # (512, 512) bf16
        lhsX = np.ascontiguousarray(
            (-2.0 * Xl.astype(np.float32)).astype(BF16).T.reshape(KCH, 128, MLOC))
        sqb = (sq32[r0:r0 + MLOC] + np.float32(C0)).reshape(MCH, 128, 1)
        uu = np.ascontiguousarray(UU[r0:r0 + MLOC].reshape(MCH, 128, 24))
        in_maps.append({
            "lhsX": lhsX,
            "rhsX": rhsX,
            "sqj": sqj,
            "sqb": sqb.astype(np.float32),
            "uu": uu,
        })
    return in_maps


def finish(results, ds, y, n_classes, n_domains):
    ds = np.asarray(ds).astype(np.int64)
    y = np.asarray(y).astype(np.int64)
    n_classes = int(n_classes)
    n_domains = int(n_domains)
    combo = (y * 3 + ds).astype(np.int64)
    jj = np.arange(BS)

    sa_sum = 0.0
    smin_sum = 0.0
    for c in range(NCORES):
        T = np.asarray(results[c]["out"], dtype=np.float64)   # (44, 4096)
        sa_sum += T[0:12][combo, jj].sum()
        smin_sum += T[32:44][combo, jj].sum()

    # exact pair count for the s mask
    cnt = np.bincount(combo, minlength=12).astype(np.float64)
    cc = np.arange(12) // 3
    aa = np.arange(12) % 3
    Ms = ((cc[:, None] < cc[None, :]) & (aa[:, None] < aa[None, :])).astype(np.float64)
    n_pairs_s = cnt @ Ms @ cnt

    n_sa = n_classes * (n_domains * (n_domains - 1) // 2)
    n_s = (n_classes * (n_classes - 1) // 2) * (n_domains * (n_domains - 1) // 2)
    sa_loss = 0.5 * sa_sum / n_sa
    s_loss = 0.5 * (n_pairs_s - smin_sum) / n_s
    return np.array([sa_loss, s_loss], dtype=np.float32)


def run_device(in_maps, trace=False, **kw):
    nc = build_program()
    return run_bass_kernel_spmd(nc, in_maps, core_ids=list(range(NCORES)),
                                trace=trace, **kw)


def kernel(X, ds, y, n_classes, n_domains):
    in_maps = prepare_inputs(X, ds, y)
    res = run_device(in_maps)
    return finish(res.results, ds, y, n_classes, n_domains)
